# revision 22
# baseline (speedup 1.0000x reference)
"""Category-specific linear (MoE-style routed batched matmul) on 8 trn2 cores.

out[b, s, h] = sum_i x[b, s, i] * W[cat_ids[b], i, h] + bias[cat_ids[b], h]

Shapes (hardcoded): x (32, 512, 1024) f32, cat_ids (32,) int, W (16, 1024, 4096)
f32, b (16, 4096) f32 -> out (32, 512, 4096) f32.

Strategy: data-parallel over batch, 4 batches per core, with host-side routing
that always packs one same-category PAIR of batches plus two singles per core
(slot capacities [2, 1, 1] batches). With 32 batches over 16 categories there
are always >= (32 - 16)/2 = 8 disjoint same-category pairs, so this packing is
feasible for ANY cat_ids. Each core then loads only 3 weight matrices (24 MB
in f16) instead of 4, keeping the kernel compute-bound.

v2 schedule notes (from trace analysis of the v1 267 us baseline):
  - PE steady state was already at the 216 ns/MM issue-rate roofline; the
    ~45 us of slack was startup (12 us serialized warmup chain + W-ring
    starvation while the bias broadcast DMA burned ~150 GB/s) and tail.
  - bias now rides the scalar HWDGE ring in f16 (0.5 MB/slot, after xt),
    leaving the sync ring 100% for W from t=0; gpsimd SWDGE is unused.
  - xt is host-packed kt-major so each batch is one DMA with 8 KB
    contiguous per-partition rows (was 1 KB packets).
  - warmup is 20 short (N=256) matmuls round-robin over all 8 PSUM banks
    (pipelined, ~220 ns each warm / 430 cold) instead of 22 serialized
    same-bank matmuls; it just bridges HAM's ~3.4 us busy window until the
    first xt+W tiles land.
  - fp8 was considered and rejected: max-rel-err would be ~5e-2 > 2e-2 gate.

Per core (slot-major):
  for slot s in [A(2 batches), B(1), C(1)]:
    for half (2 x 2048 cols):
      stream W[s]-half as 8 k-tiles [128, 2048] f16 on the sync HWDGE ring
      for m over the slot's 128-sample tiles (8 for A, 4 for B/C):
        for kt(8): 4 matmuls (2 psum tiles [128,1024] x 2 banks), accum kt
        evict psum + bias (2 DVE adds) -> out tile, DMA to out (scalar ring)
"""

import numpy as np

import concourse.bacc as bacc
import concourse.mybir as mybir
import concourse.bass as bass
import concourse.tile as tile
from concourse.bass_utils import run_bass_kernel_spmd

N_CORES = 8
B, S, K, H = 32, 512, 1024, 4096
BPC = B // N_CORES          # batches per core
P = 128                     # partitions
KT = K // P                 # k tiles (8)
MT = S // P                 # sample tiles per batch (4)
NHALF = 2                   # n halves
NH = H // NHALF             # cols per half (2048)
SLOT_BATCHES = (2, 1, 1)    # batches per weight slot
NSLOT = len(SLOT_BATCHES)
N_WARM = 10                 # warmup matmuls (N=256, round robin over 8 banks)

_COMPILED = None


def _build():
    nc = bacc.Bacc("TRN2", target_bir_lowering=False, debug=False)
    f32 = mybir.dt.float32
    f16 = mybir.dt.float16

    # xt: per batch, partition p holds x[b, :, kt*128+p] for kt=0..7, i.e.
    # row layout [kt, m] (8 KB contiguous per partition row).
    xt_ap = nc.dram_tensor("xt", [BPC, P, KT * S], f16, kind="ExternalInput").ap()
    # w: [slot, half, kt, p, n] so each (slot, half, kt) tile is [128, 2048]
    # with 4 KB contiguous per-partition rows.
    w_ap = nc.dram_tensor(
        "w", [NSLOT, NHALF, KT, P, NH], f16, kind="ExternalInput"
    ).ap()
    # Phase-0 W (slot A, cols 0:1024) duplicated in kt-pair layout: tile t
    # holds kt=2t,2t+1 as [128, 2048] with 4 KB contiguous rows (2x larger
    # DMA packets than the 2 KB rows a quarter-wide slice of `w` would give).
    w0_ap = nc.dram_tensor("w0", [KT // 2, P, 2048], f16, kind="ExternalInput").ap()
    bias_ap = nc.dram_tensor("bias", [NSLOT, H], f16, kind="ExternalInput").ap()
    out_ap = nc.dram_tensor("out", [BPC, S, H], f32, kind="ExternalOutput").ap()

    with tile.TileContext(nc) as tc:
        with (
            tc.tile_pool(name="xt_pool", bufs=4) as xt_pool,
            tc.tile_pool(name="w_pool", bufs=16) as w_pool,
            tc.tile_pool(name="bias_pool", bufs=2) as bias_pool,
            tc.tile_pool(name="out_pool", bufs=4) as out_pool,
            tc.tile_pool(name="ps_pool", bufs=4, space="PSUM") as ps_pool,
        ):
            # Allocate xt and bias tiles up front. xt_b0 gates the very first
            # matmul, and the sync ring wakes ~4 us before the scalar ring,
            # so xt_b0 rides the sync ring ahead of all W. Everything not
            # needed in the first ~30 us is deferred via tile_wait_until so
            # the scheduler can't hoist it into the startup window.
            xt_ts = [
                xt_pool.tile([P, KT * S], f16, name="xt_t", tag="xt")
                for _ in range(BPC)
            ]
            bias_ts = [
                bias_pool.tile([P, H], f16, name="bias_t") for _ in range(NSLOT)
            ]

            def dma_xt(b, eng):
                eng.dma_start(xt_ts[b][:], xt_ap[b])

            def dma_bias(s, eng):
                bias_src = bias_ap[s]
                eng.dma_start(
                    bias_ts[s][:],
                    bass.AP(
                        tensor=bias_src.tensor,
                        offset=bias_src.offset,
                        ap=[[0, P]] + list(bias_src.ap),
                    ),
                )

            # Startup choreography. The sync ring wakes ~4 us before the
            # scalar ring, so the most critical bytes lead it:
            #   sync:   w0(kt01), xt_b0[kt0-3], xt_b0[kt4-7], w0(kt23),
            #           bias_A, then all later-phase W
            #   scalar: w0(kt45), w0(kt67), then deferred xt/bias + stores
            # Everything not needed before ~25 us is pushed out of the window
            # via tile_wait_until so the scheduler can't hoist it forward.
            hx = KT * S // 2
            w0_tiles = [
                w_pool.tile([P, 2048], f16, tag="w", name=f"w0_{t}")
                for t in range(KT // 2)
            ]
            nc.sync.dma_start(w0_tiles[0][:], w0_ap[0])
            nc.sync.dma_start(xt_ts[0][:, 0:hx], xt_ap[0][:, 0:hx])
            nc.sync.dma_start(xt_ts[0][:, hx:], xt_ap[0][:, hx:])
            nc.sync.dma_start(w0_tiles[1][:], w0_ap[1])
            dma_bias(0, nc.sync)
            nc.scalar.dma_start(w0_tiles[2][:], w0_ap[2])
            nc.scalar.dma_start(w0_tiles[3][:], w0_ap[3])
            with tc.tile_wait_until(0.012):
                dma_xt(1, nc.scalar)
            with tc.tile_wait_until(0.030):
                dma_xt(2, nc.scalar)
            with tc.tile_wait_until(0.040):
                dma_xt(3, nc.scalar)
            with tc.tile_wait_until(0.050):
                dma_bias(1, nc.scalar)
            with tc.tile_wait_until(0.070):
                dma_bias(2, nc.scalar)

            # Warm up the PE (HAM un-throttle) while the first DMAs land:
            # short matmuls on memset tiles, round-robin over all 8 PSUM banks
            # so they pipeline at issue rate instead of serializing on one
            # bank. Results read once so DCE keeps them.
            warm_x = xt_pool.tile([P, P], f16, name="warm_x", tag="warm")
            warm_w = w_pool.tile([P, 256], f16, tag="warmw", name="warm_w")
            nc.vector.memset(warm_x[:], 0.0)
            nc.vector.memset(warm_w[:], 0.0)
            warm_ps = [
                ps_pool.tile([P, 1024], f32, tag="ps", name="warm_ps")
                for _ in range(4)
            ]
            for i in range(N_WARM):
                t = warm_ps[(i // 2) % 4]
                col = 512 * (i % 2)
                nc.tensor.matmul(
                    t[:, col : col + 256], warm_x[:], warm_w[:],
                    start=True, stop=True, skip_group_check=True,
                )
            warm_out = out_pool.tile([P, 16], f32, name="warm_out", tag="warmo")
            for i in range(4):
                nc.vector.tensor_copy(warm_out[:, 4 * i : 4 * i + 4], warm_ps[i][:, 0:4])

            # Phases: slot A (2 batches) runs 4 quarter-width (1024-col)
            # phases so the DMA-gated first m-sweep needs only 2 MB of W;
            # single-batch slots B/C run 2 half-width (2048-col) phases.
            phases = []
            bi0 = 0
            for s in range(NSLOT):
                nb = SLOT_BATCHES[s]
                ncols = 1024 if s == 0 else NH
                for col0 in range(0, H, ncols):
                    phases.append((s, bi0, nb, col0, ncols))
                bi0 += nb

            for pi, (s, bi0, nb, col0, ncols) in enumerate(phases):
                half, off = divmod(col0, NH)
                if pi == 0:
                    def w_slice(kt, n4):
                        base = (kt % 2) * 1024 + n4 * 512
                        return w0_tiles[kt // 2][:, base : base + 512]
                else:
                    w_tiles = []
                    for kt in range(KT):
                        w_t = w_pool.tile([P, ncols], f16, tag="w", name="w_t")
                        nc.sync.dma_start(
                            w_t[:], w_ap[s, half, kt, :, off : off + ncols]
                        )
                        w_tiles.append(w_t)

                    def w_slice(kt, n4, w_tiles=w_tiles):
                        return w_tiles[kt][:, n4 * 512 : (n4 + 1) * 512]
                nps = ncols // 1024
                for m in range(nb * MT):
                    b, mm = divmod(m, MT)
                    last_iter = pi == len(phases) - 1 and m == nb * MT - 1
                    ps = [
                        ps_pool.tile([P, 1024], f32, tag="ps", name="ps")
                        for _ in range(nps)
                    ]
                    if last_iter:
                        # n-major so ps0 finishes early: evict + store it
                        # while ps1's matmuls still run (shorter tail).
                        mm_order = [
                            (kt, n4) for n4 in range(2 * nps) for kt in range(KT)
                        ]
                    else:
                        mm_order = [
                            (kt, n4) for kt in range(KT) for n4 in range(2 * nps)
                        ]
                    for kt, n4 in mm_order:
                        lhsT = xt_ts[bi0 + b][
                            :, kt * S + mm * P : kt * S + (mm + 1) * P
                        ]
                        nc.tensor.matmul(
                            ps[n4 // 2][:, (n4 % 2) * 512 : (n4 % 2) * 512 + 512],
                            lhsT,
                            w_slice(kt, n4),
                            start=(kt == 0),
                            stop=(kt == KT - 1),
                        )
                    out_t = out_pool.tile([P, ncols], f32, tag="o", name="out_t")
                    if last_iter:
                        # Fine-grained evict pipeline: each 512-chunk's add
                        # starts as soon as its bank's accumulation (n-major
                        # order) completes; stores go out per 1024 cols
                        # (4 KB rows DMA much faster than 2 KB).
                        for q in range(2 * nps):
                            nc.vector.tensor_add(
                                out_t[:, q * 512 : (q + 1) * 512],
                                ps[q // 2][:, (q % 2) * 512 : (q % 2) * 512 + 512],
                                bias_ts[s][:, col0 + q * 512 : col0 + (q + 1) * 512],
                            )
                            if q % 2 == 1:
                                nc.scalar.dma_start(
                                    out_ap[
                                        bi0 + b,
                                        mm * P : (mm + 1) * P,
                                        col0 + (q - 1) * 512 : col0 + (q + 1) * 512,
                                    ],
                                    out_t[:, (q - 1) * 512 : (q + 1) * 512],
                                )
                    else:
                        for h2 in range(nps):
                            nc.vector.tensor_add(
                                out_t[:, h2 * 1024 : (h2 + 1) * 1024],
                                ps[h2][:],
                                bias_ts[s][:, col0 + h2 * 1024 : col0 + (h2 + 1) * 1024],
                            )
                        nc.scalar.dma_start(
                            out_ap[
                                bi0 + b,
                                mm * P : (mm + 1) * P,
                                col0 : col0 + ncols,
                            ],
                            out_t[:],
                        )
    nc.compile()
    return nc


def _get_compiled():
    global _COMPILED
    if _COMPILED is None:
        _COMPILED = _build()
    return _COMPILED


def _pack(cat_ids):
    """Assign batches to cores with slot capacities [2,1,1] per core.

    Returns per-core (idx, slot_cats): idx = 4 batch indices ordered
    [pair0, pair1, single_b, single_c]; slot_cats = categories for the 3 slots.
    Always feasible: #disjoint same-cat pairs = (32 - #odd-count cats)/2 >= 8.
    """
    cat_ids = np.asarray(cat_ids)
    by_cat = {}
    for i, c in enumerate(cat_ids.tolist()):
        by_cat.setdefault(c, []).append(i)
    pairs = []
    singles = []
    for c, idxs in sorted(by_cat.items()):
        n = len(idxs)
        for j in range(n // 2):
            pairs.append((c, idxs[2 * j], idxs[2 * j + 1]))
        if n % 2:
            singles.append((c, idxs[-1]))
    assert len(pairs) >= N_CORES, "impossible: <8 same-cat pairs among 32 batches"
    core_pairs = pairs[:N_CORES]
    # leftovers: extra pairs flatten into singles
    for c, i, j in pairs[N_CORES:]:
        singles.append((c, i))
        singles.append((c, j))
    assert len(singles) == 2 * N_CORES
    cores = []
    for ci in range(N_CORES):
        c, i, j = core_pairs[ci]
        (cb, ib), (cc, ic) = singles[2 * ci], singles[2 * ci + 1]
        cores.append(([i, j, ib, ic], [c, cb, cc]))
    return cores


def _host_pack_xt(xb):
    """x batches (n, 512, 1024) f32 -> (n, 128, KT*S) f16, kt-major rows.

    xt[b, p, kt*512 + m] = x[b, m, kt*128 + p]
    """
    n = xb.shape[0]
    xt = xb.astype(np.float16).transpose(0, 2, 1)          # (n, K, S)
    xt = xt.reshape(n, KT, P, S).transpose(0, 2, 1, 3)     # (n, P, KT, S)
    return np.ascontiguousarray(xt.reshape(n, P, KT * S))


def _host_pack_w(Wsel):
    """W slots (3, 1024, 4096) f32 -> (3, NHALF, KT, P, NH) f16.

    w[s, h, kt, p, j] = W[s, kt*128 + p, h*2048 + j]
    """
    w = Wsel.astype(np.float16).reshape(NSLOT, KT, P, NHALF, NH)
    return np.ascontiguousarray(w.transpose(0, 3, 1, 2, 4))


def _host_pack_w0(W0):
    """Slot-A W cols 0:1024 (1024, 1024) f32 -> (KT//2, P, 2048) f16 kt-pairs.

    w0[t, p, i*1024 + j] = W0[(2t + i)*128 + p, j]
    """
    w0 = W0[:, 0:1024].astype(np.float16).reshape(KT // 2, 2, P, 1024)
    return np.ascontiguousarray(w0.transpose(0, 2, 1, 3).reshape(KT // 2, P, 2048))


def run_sharded(x, cat_ids, W, b, trace=False, **spmd_kwargs):
    """Shard, run on 8 cores, unshard. Returns (out, BassKernelResults)."""
    x = np.ascontiguousarray(np.asarray(x), dtype=np.float32)
    cat_ids = np.asarray(cat_ids).astype(np.int64)
    W = np.ascontiguousarray(np.asarray(W), dtype=np.float32)
    b = np.ascontiguousarray(np.asarray(b), dtype=np.float32)

    nc = _get_compiled()
    cores = _pack(cat_ids)

    in_maps = []
    for idx, slot_cats in cores:
        in_maps.append(
            {
                "xt": _host_pack_xt(x[idx]),
                "w": _host_pack_w(W[slot_cats]),
                "w0": _host_pack_w0(W[slot_cats[0]]),
                "bias": b[slot_cats].astype(np.float16),
            }
        )

    res = run_bass_kernel_spmd(
        nc, in_maps, list(range(N_CORES)), trace=trace, **spmd_kwargs
    )

    out = np.empty((B, S, H), dtype=np.float32)
    for c, (idx, _) in enumerate(cores):
        out[idx] = res.results[c]["out"]
    return out, res


def kernel(x, cat_ids, W, b):
    out, _ = run_sharded(x, cat_ids, W, b)
    return out


# revision 26
# speedup vs baseline: 1.0029x; 1.0029x over previous
"""Category-specific linear (MoE-style routed batched matmul) on 8 trn2 cores.

out[b, s, h] = sum_i x[b, s, i] * W[cat_ids[b], i, h] + bias[cat_ids[b], h]

Shapes (hardcoded): x (32, 512, 1024) f32, cat_ids (32,) int, W (16, 1024, 4096)
f32, b (16, 4096) f32 -> out (32, 512, 4096) f32.

Strategy: data-parallel over batch, 4 batches per core, with host-side routing
that always packs one same-category PAIR of batches plus two singles per core
(slot capacities [2, 1, 1] batches). With 32 batches over 16 categories there
are always >= (32 - 16)/2 = 8 disjoint same-category pairs, so this packing is
feasible for ANY cat_ids. Each core then loads only 3 weight matrices (24 MB
in f16) instead of 4, keeping the kernel compute-bound.

v2 schedule notes (from trace analysis of the v1 267 us baseline):
  - PE steady state was already at the 216 ns/MM issue-rate roofline; the
    ~45 us of slack was startup (12 us serialized warmup chain + W-ring
    starvation while the bias broadcast DMA burned ~150 GB/s) and tail.
  - bias now rides the scalar HWDGE ring in f16 (0.5 MB/slot, after xt),
    leaving the sync ring 100% for W from t=0; gpsimd SWDGE is unused.
  - xt is host-packed kt-major so each batch is one DMA with 8 KB
    contiguous per-partition rows (was 1 KB packets).
  - warmup is 20 short (N=256) matmuls round-robin over all 8 PSUM banks
    (pipelined, ~220 ns each warm / 430 cold) instead of 22 serialized
    same-bank matmuls; it just bridges HAM's ~3.4 us busy window until the
    first xt+W tiles land.
  - fp8 was considered and rejected: max-rel-err would be ~5e-2 > 2e-2 gate.

Per core (slot-major):
  for slot s in [A(2 batches), B(1), C(1)]:
    for half (2 x 2048 cols):
      stream W[s]-half as 8 k-tiles [128, 2048] f16 on the sync HWDGE ring
      for m over the slot's 128-sample tiles (8 for A, 4 for B/C):
        for kt(8): 4 matmuls (2 psum tiles [128,1024] x 2 banks), accum kt
        evict psum + bias (2 DVE adds) -> out tile, DMA to out (scalar ring)
"""

import numpy as np

import concourse.bacc as bacc
import concourse.mybir as mybir
import concourse.bass as bass
import concourse.tile as tile
from concourse.bass_utils import run_bass_kernel_spmd

N_CORES = 8
B, S, K, H = 32, 512, 1024, 4096
BPC = B // N_CORES          # batches per core
P = 128                     # partitions
KT = K // P                 # k tiles (8)
MT = S // P                 # sample tiles per batch (4)
NHALF = 2                   # n halves
NH = H // NHALF             # cols per half (2048)
SLOT_BATCHES = (2, 1, 1)    # batches per weight slot
NSLOT = len(SLOT_BATCHES)
N_WARM = 10                 # warmup matmuls (N=256, round robin over 8 banks)

_COMPILED = None


def _build():
    nc = bacc.Bacc("TRN2", target_bir_lowering=False, debug=False)
    f32 = mybir.dt.float32
    f16 = mybir.dt.float16

    # xt: per batch, partition p holds x[b, :, .*128+p] in m-tile-major
    # layout [mm, kt, j] (8 KB contiguous per partition row; each m-tile's
    # lhsT data is a 2 KB sub-run so xt_b0 can stream in m-tile chunks).
    xt_ap = nc.dram_tensor("xt", [BPC, P, KT * S], f16, kind="ExternalInput").ap()
    # w: [slot, half, kt, p, n] so each (slot, half, kt) tile is [128, 2048]
    # with 4 KB contiguous per-partition rows.
    w_ap = nc.dram_tensor(
        "w", [NSLOT, NHALF, KT, P, NH], f16, kind="ExternalInput"
    ).ap()
    # Phase-0 W (slot A, cols 0:1024) duplicated in kt-pair layout: tile t
    # holds kt=2t,2t+1 as [128, 2048] with 4 KB contiguous rows (2x larger
    # DMA packets than the 2 KB rows a quarter-wide slice of `w` would give).
    w0_ap = nc.dram_tensor("w0", [KT // 2, P, 2048], f16, kind="ExternalInput").ap()
    bias_ap = nc.dram_tensor("bias", [NSLOT, H], f16, kind="ExternalInput").ap()
    out_ap = nc.dram_tensor("out", [BPC, S, H], f32, kind="ExternalOutput").ap()

    with tile.TileContext(nc) as tc:
        with (
            tc.tile_pool(name="xt_pool", bufs=4) as xt_pool,
            tc.tile_pool(name="w_pool", bufs=16) as w_pool,
            tc.tile_pool(name="bias_pool", bufs=2) as bias_pool,
            tc.tile_pool(name="out_pool", bufs=4) as out_pool,
            tc.tile_pool(name="ps_pool", bufs=4, space="PSUM") as ps_pool,
        ):
            # Allocate xt and bias tiles up front. xt_b0 gates the very first
            # matmul, and the sync ring wakes ~4 us before the scalar ring,
            # so xt_b0 rides the sync ring ahead of all W. Everything not
            # needed in the first ~30 us is deferred via tile_wait_until so
            # the scheduler can't hoist it into the startup window.
            xt_ts = [
                xt_pool.tile([P, KT * S], f16, name="xt_t", tag="xt")
                for _ in range(BPC)
            ]
            bias_ts = [
                bias_pool.tile([P, H], f16, name="bias_t") for _ in range(NSLOT)
            ]

            def dma_xt(b, eng):
                eng.dma_start(xt_ts[b][:], xt_ap[b])

            def dma_bias(s, eng):
                bias_src = bias_ap[s]
                eng.dma_start(
                    bias_ts[s][:],
                    bass.AP(
                        tensor=bias_src.tensor,
                        offset=bias_src.offset,
                        ap=[[0, P]] + list(bias_src.ap),
                    ),
                )

            # Startup choreography. The sync ring wakes ~4 us before the
            # scalar ring, so the most critical bytes lead it:
            #   sync:   w0(kt01), xt_b0[kt0-3], xt_b0[kt4-7], w0(kt23),
            #           bias_A, then all later-phase W
            #   scalar: w0(kt45), w0(kt67), then deferred xt/bias + stores
            # Everything not needed before ~25 us is pushed out of the window
            # via tile_wait_until so the scheduler can't hoist it forward.
            w0_tiles = [
                w_pool.tile([P, 2048], f16, tag="w", name=f"w0_{t}")
                for t in range(KT // 2)
            ]
            MTW = KT * P  # xt row elems per m-tile chunk (1024)
            nc.sync.dma_start(w0_tiles[0][:], w0_ap[0])
            nc.sync.dma_start(xt_ts[0][:, 0:MTW], xt_ap[0][:, 0:MTW])
            nc.sync.dma_start(xt_ts[0][:, MTW : 2 * MTW], xt_ap[0][:, MTW : 2 * MTW])
            nc.sync.dma_start(w0_tiles[1][:], w0_ap[1])
            nc.sync.dma_start(xt_ts[0][:, 2 * MTW :], xt_ap[0][:, 2 * MTW :])
            dma_bias(0, nc.sync)
            nc.scalar.dma_start(w0_tiles[2][:], w0_ap[2])
            nc.scalar.dma_start(w0_tiles[3][:], w0_ap[3])
            with tc.tile_wait_until(0.012):
                dma_xt(1, nc.scalar)
            with tc.tile_wait_until(0.030):
                dma_xt(2, nc.scalar)
            with tc.tile_wait_until(0.040):
                dma_xt(3, nc.scalar)
            with tc.tile_wait_until(0.050):
                dma_bias(1, nc.scalar)
            with tc.tile_wait_until(0.070):
                dma_bias(2, nc.scalar)

            # Warm up the PE (HAM un-throttle) while the first DMAs land:
            # short matmuls on memset tiles, round-robin over all 8 PSUM banks
            # so they pipeline at issue rate instead of serializing on one
            # bank. Results read once so DCE keeps them.
            warm_x = xt_pool.tile([P, P], f16, name="warm_x", tag="warm")
            warm_w = w_pool.tile([P, 256], f16, tag="warmw", name="warm_w")
            nc.vector.memset(warm_x[:], 0.0)
            nc.vector.memset(warm_w[:], 0.0)
            warm_ps = [
                ps_pool.tile([P, 1024], f32, tag="ps", name="warm_ps")
                for _ in range(4)
            ]
            for i in range(N_WARM):
                t = warm_ps[(i // 2) % 4]
                col = 512 * (i % 2)
                nc.tensor.matmul(
                    t[:, col : col + 256], warm_x[:], warm_w[:],
                    start=True, stop=True, skip_group_check=True,
                )
            warm_out = out_pool.tile([P, 16], f32, name="warm_out", tag="warmo")
            for i in range(4):
                nc.vector.tensor_copy(warm_out[:, 4 * i : 4 * i + 4], warm_ps[i][:, 0:4])

            # Phases: slot A (2 batches) runs 4 quarter-width (1024-col)
            # phases so the DMA-gated first m-sweep needs only 2 MB of W;
            # single-batch slots B/C run 2 half-width (2048-col) phases.
            phases = []
            bi0 = 0
            for s in range(NSLOT):
                nb = SLOT_BATCHES[s]
                ncols = 1024 if s == 0 else NH
                for col0 in range(0, H, ncols):
                    phases.append((s, bi0, nb, col0, ncols))
                bi0 += nb

            for pi, (s, bi0, nb, col0, ncols) in enumerate(phases):
                half, off = divmod(col0, NH)
                if pi == 0:
                    def w_slice(kt, n4):
                        base = (kt % 2) * 1024 + n4 * 512
                        return w0_tiles[kt // 2][:, base : base + 512]
                else:
                    w_tiles = []
                    for kt in range(KT):
                        w_t = w_pool.tile([P, ncols], f16, tag="w", name="w_t")
                        nc.sync.dma_start(
                            w_t[:], w_ap[s, half, kt, :, off : off + ncols]
                        )
                        w_tiles.append(w_t)

                    def w_slice(kt, n4, w_tiles=w_tiles):
                        return w_tiles[kt][:, n4 * 512 : (n4 + 1) * 512]
                nps = ncols // 1024
                for m in range(nb * MT):
                    b, mm = divmod(m, MT)
                    last_iter = pi == len(phases) - 1 and m == nb * MT - 1
                    ps = [
                        ps_pool.tile([P, 1024], f32, tag="ps", name="ps")
                        for _ in range(nps)
                    ]
                    if last_iter:
                        # n-major so ps0 finishes early: evict + store it
                        # while ps1's matmuls still run (shorter tail).
                        mm_order = [
                            (kt, n4) for n4 in range(2 * nps) for kt in range(KT)
                        ]
                    else:
                        mm_order = [
                            (kt, n4) for kt in range(KT) for n4 in range(2 * nps)
                        ]
                    for kt, n4 in mm_order:
                        lhsT = xt_ts[bi0 + b][
                            :, mm * KT * P + kt * P : mm * KT * P + (kt + 1) * P
                        ]
                        nc.tensor.matmul(
                            ps[n4 // 2][:, (n4 % 2) * 512 : (n4 % 2) * 512 + 512],
                            lhsT,
                            w_slice(kt, n4),
                            start=(kt == 0),
                            stop=(kt == KT - 1),
                        )
                    out_t = out_pool.tile([P, ncols], f32, tag="o", name="out_t")
                    if last_iter:
                        # Fine-grained evict pipeline: each 512-chunk's add
                        # starts as soon as its bank's accumulation (n-major
                        # order) completes; stores go out per 1024 cols
                        # (4 KB rows DMA much faster than 2 KB).
                        for q in range(2 * nps):
                            nc.vector.tensor_add(
                                out_t[:, q * 512 : (q + 1) * 512],
                                ps[q // 2][:, (q % 2) * 512 : (q % 2) * 512 + 512],
                                bias_ts[s][:, col0 + q * 512 : col0 + (q + 1) * 512],
                            )
                            if q % 2 == 1:
                                nc.scalar.dma_start(
                                    out_ap[
                                        bi0 + b,
                                        mm * P : (mm + 1) * P,
                                        col0 + (q - 1) * 512 : col0 + (q + 1) * 512,
                                    ],
                                    out_t[:, (q - 1) * 512 : (q + 1) * 512],
                                )
                    else:
                        for h2 in range(nps):
                            nc.vector.tensor_add(
                                out_t[:, h2 * 1024 : (h2 + 1) * 1024],
                                ps[h2][:],
                                bias_ts[s][:, col0 + h2 * 1024 : col0 + (h2 + 1) * 1024],
                            )
                        nc.scalar.dma_start(
                            out_ap[
                                bi0 + b,
                                mm * P : (mm + 1) * P,
                                col0 : col0 + ncols,
                            ],
                            out_t[:],
                        )
    nc.compile()
    return nc


def _get_compiled():
    global _COMPILED
    if _COMPILED is None:
        _COMPILED = _build()
    return _COMPILED


def _pack(cat_ids):
    """Assign batches to cores with slot capacities [2,1,1] per core.

    Returns per-core (idx, slot_cats): idx = 4 batch indices ordered
    [pair0, pair1, single_b, single_c]; slot_cats = categories for the 3 slots.
    Always feasible: #disjoint same-cat pairs = (32 - #odd-count cats)/2 >= 8.
    """
    cat_ids = np.asarray(cat_ids)
    by_cat = {}
    for i, c in enumerate(cat_ids.tolist()):
        by_cat.setdefault(c, []).append(i)
    pairs = []
    singles = []
    for c, idxs in sorted(by_cat.items()):
        n = len(idxs)
        for j in range(n // 2):
            pairs.append((c, idxs[2 * j], idxs[2 * j + 1]))
        if n % 2:
            singles.append((c, idxs[-1]))
    assert len(pairs) >= N_CORES, "impossible: <8 same-cat pairs among 32 batches"
    core_pairs = pairs[:N_CORES]
    # leftovers: extra pairs flatten into singles
    for c, i, j in pairs[N_CORES:]:
        singles.append((c, i))
        singles.append((c, j))
    assert len(singles) == 2 * N_CORES
    cores = []
    for ci in range(N_CORES):
        c, i, j = core_pairs[ci]
        (cb, ib), (cc, ic) = singles[2 * ci], singles[2 * ci + 1]
        cores.append(([i, j, ib, ic], [c, cb, cc]))
    return cores


def _host_pack_xt(xb):
    """x batches (n, 512, 1024) f32 -> (n, 128, KT*S) f16, m-tile-major rows.

    xt[b, p, mm*1024 + kt*128 + j] = x[b, mm*128 + j, kt*128 + p]
    """
    n = xb.shape[0]
    xt = xb.astype(np.float16).transpose(0, 2, 1)           # (n, K, S)
    xt = xt.reshape(n, KT, P, MT, P)                        # (n, kt, p, mm, j)
    xt = xt.transpose(0, 2, 3, 1, 4)                        # (n, p, mm, kt, j)
    return np.ascontiguousarray(xt.reshape(n, P, KT * S))


def _host_pack_w(Wsel):
    """W slots (3, 1024, 4096) f32 -> (3, NHALF, KT, P, NH) f16.

    w[s, h, kt, p, j] = W[s, kt*128 + p, h*2048 + j]
    """
    w = Wsel.astype(np.float16).reshape(NSLOT, KT, P, NHALF, NH)
    return np.ascontiguousarray(w.transpose(0, 3, 1, 2, 4))


def _host_pack_w0(W0):
    """Slot-A W cols 0:1024 (1024, 1024) f32 -> (KT//2, P, 2048) f16 kt-pairs.

    w0[t, p, i*1024 + j] = W0[(2t + i)*128 + p, j]
    """
    w0 = W0[:, 0:1024].astype(np.float16).reshape(KT // 2, 2, P, 1024)
    return np.ascontiguousarray(w0.transpose(0, 2, 1, 3).reshape(KT // 2, P, 2048))


def run_sharded(x, cat_ids, W, b, trace=False, **spmd_kwargs):
    """Shard, run on 8 cores, unshard. Returns (out, BassKernelResults)."""
    x = np.ascontiguousarray(np.asarray(x), dtype=np.float32)
    cat_ids = np.asarray(cat_ids).astype(np.int64)
    W = np.ascontiguousarray(np.asarray(W), dtype=np.float32)
    b = np.ascontiguousarray(np.asarray(b), dtype=np.float32)

    nc = _get_compiled()
    cores = _pack(cat_ids)

    in_maps = []
    for idx, slot_cats in cores:
        in_maps.append(
            {
                "xt": _host_pack_xt(x[idx]),
                "w": _host_pack_w(W[slot_cats]),
                "w0": _host_pack_w0(W[slot_cats[0]]),
                "bias": b[slot_cats].astype(np.float16),
            }
        )

    res = run_bass_kernel_spmd(
        nc, in_maps, list(range(N_CORES)), trace=trace, **spmd_kwargs
    )

    out = np.empty((B, S, H), dtype=np.float32)
    for c, (idx, _) in enumerate(cores):
        out[idx] = res.results[c]["out"]
    return out, res


def kernel(x, cat_ids, W, b):
    out, _ = run_sharded(x, cat_ids, W, b)
    return out
